# revision 1
# baseline (speedup 1.0000x reference)
"""MultiHeadEMA on 8 Trainium2 NeuronCores.

Strategy
--------
Channel-sharded: embed_dim=1024 -> 8 slices of 128 channels (= SBUF
partitions), one per core. The reference's FFT conv is exactly an order-2 IIR
    y_n[l] = q_n y_n[l-1] + x[l],   out = silu(c0 y0 + c1 y1 + omega x)
computed with `tensor_tensor_scan` on the vector engine.

The DVE scan runs at ~2.1 cyc/elem, so the recurrence is decimated by 4:
    Y_n[j] = y_n[4j] satisfies  Y_n[j] = q_n^4 Y_n[j-1] + u_n[j]
    u_n[j] = x[4j] + q_n x[4j-1] + q_n^2 x[4j-2] + q_n^3 x[4j-3]
u_n is built by accumulating diagonal matmuls (tensor engine, bf16) into
PSUM from contiguous phase blocks of x (deinterleaved and pre-shifted on the
host — a strided matmul rhs halves PE throughput). The scan reads u straight
from PSUM at 1/4 length. Phases y[4j+r] are never materialized: the outputs
    pre_r = c0 y0[4j+r] + c1 y1[4j+r] + w x[4j+r]
expand into diagonal matmuls over (Y0, Y1, phase blocks of x) with
per-channel coefficients (c_n q_n^r, sums), accumulated in PSUM, then one
Silu per 1024 columns evacuates PSUM -> SBUF (phase-major output, host
re-interleaves). Interior is bf16 (fp32 PSUM accumulation, fp32 scan state,
exact fp32 decay factors).

Engine balance at 8 concurrent cores: the chip power governor caps matmuls
at ~379ns (vs 216ns single-core) and punishes load added to the vector
engine (scans degrade), so the design keeps DVE scan-only and feeds the
tensor engine one dense contiguous stream.
"""

import numpy as np
import ml_dtypes

import concourse.bass as bass
import concourse.bacc as bacc
import concourse.tile as tile
from concourse import mybir
from concourse.bass_utils import run_bass_kernel_spmd

SEQ_LEN, BSZ, EMBED_DIM, NDIM = 4096, 4, 1024, 2
N_CORES = 8
D_PER = EMBED_DIM // N_CORES  # 128 channels/core = full SBUF partitions
SCALE = (1.0 / NDIM) ** 0.5
DEC = 4                   # decimation factor
J = SEQ_LEN // DEC        # decimated length 1024
CH = 512                  # matmul chunk (one fp32 PSUM bank)
NG = J // CH              # j-groups per slab (2)
F32 = mybir.dt.float32
BF16 = mybir.dt.bfloat16
AF = mybir.ActivationFunctionType
ALU = mybir.AluOpType

# x phase blocks: r = 0..3 -> x[4j+r]. Shifted u-operands x[4j-k] are read
# as contiguous offset-(-1) views of block (4-k); only STRIDED rhs is slow.
NBLK = 4


def build_bass():
    nc = bacc.Bacc(name="multihead_ema")
    x = nc.dram_tensor("x", [D_PER, BSZ, NBLK, J], BF16, kind="ExternalInput")
    # coef columns: [delta0, delta1, alpha0, alpha1, beta0, beta1, gamma0, gamma1, omega]
    coef = nc.dram_tensor("coef", [D_PER, 9], F32, kind="ExternalInput")
    eye = nc.dram_tensor("eye", [D_PER, D_PER], BF16, kind="ExternalInput")
    out = nc.dram_tensor("out", [D_PER, BSZ, DEC, J], BF16, kind="ExternalOutput")

    with tile.TileContext(nc) as tc:
        with (
            tc.tile_pool(name="const", bufs=1) as const,
            tc.tile_pool(name="xup", bufs=4) as xup,
            tc.tile_pool(name="xcp", bufs=4) as xcp,
            tc.tile_pool(name="yp", bufs=3) as yp,
            tc.tile_pool(name="op", bufs=4) as op,
            tc.tile_pool(name="ysp", bufs=3) as ysp,
            tc.tile_pool(name="psu", bufs=2, space="PSUM") as psu,
            tc.tile_pool(name="psc", bufs=2, space="PSUM") as psc,
        ):
            csb = const.tile([D_PER, 9], F32)
            nc.sync.dma_start(out=csb[:, :], in_=coef[:, :])
            eyesb = const.tile([D_PER, D_PER], BF16)
            nc.sync.dma_start(out=eyesb[:, :], in_=eye[:, :])

            # --- per-channel coefficients ([128, 1/2] fp32, trivial)
            sig = const.tile([D_PER, 4], F32)  # [p0, p1, sa0, sa1]
            nc.scalar.activation(out=sig[:, :], in_=csb[:, 0:4], func=AF.Sigmoid)
            pq = const.tile([D_PER, NDIM], F32)
            nc.vector.tensor_mul(out=pq[:, :], in0=sig[:, 0:2], in1=sig[:, 2:4])
            q = const.tile([D_PER, NDIM], F32)  # q = 1 - p*sigmoid(alpha)
            nc.scalar.activation(out=q[:, :], in_=pq[:, :], func=AF.Copy,
                                 scale=-1.0, bias=1.0)
            q2 = const.tile([D_PER, NDIM], F32)
            nc.vector.tensor_mul(out=q2[:, :], in0=q[:, :], in1=q[:, :])
            q3 = const.tile([D_PER, NDIM], F32)
            nc.vector.tensor_mul(out=q3[:, :], in0=q2[:, :], in1=q[:, :])
            q4 = const.tile([D_PER, NDIM], F32)
            nc.vector.tensor_mul(out=q4[:, :], in0=q2[:, :], in1=q2[:, :])
            c1t = const.tile([D_PER, NDIM], F32)
            nc.vector.tensor_mul(out=c1t[:, :], in0=sig[:, 0:2], in1=csb[:, 4:6])
            c2t = const.tile([D_PER, NDIM], F32)
            nc.vector.tensor_mul(out=c2t[:, :], in0=c1t[:, :], in1=csb[:, 6:8])
            cc = const.tile([D_PER, NDIM], F32)  # c_n = p beta gamma scale
            nc.scalar.mul(out=cc[:, :], in_=c2t[:, :], mul=SCALE)
            cq = const.tile([D_PER, NDIM], F32)   # c_n q_n
            nc.vector.tensor_mul(out=cq[:, :], in0=cc[:, :], in1=q[:, :])
            cq2 = const.tile([D_PER, NDIM], F32)  # c_n q_n^2
            nc.vector.tensor_mul(out=cq2[:, :], in0=cc[:, :], in1=q2[:, :])
            cq3 = const.tile([D_PER, NDIM], F32)  # c_n q_n^3
            nc.vector.tensor_mul(out=cq3[:, :], in0=cc[:, :], in1=q3[:, :])
            csum = const.tile([D_PER, 1], F32)    # c0 + c1 + w
            nc.vector.tensor_add(out=csum[:, :], in0=cc[:, 0:1], in1=cc[:, 1:2])
            nc.vector.tensor_add(out=csum[:, :], in0=csum[:, :], in1=csb[:, 8:9])
            cqs = const.tile([D_PER, 1], F32)     # c0 q0 + c1 q1
            nc.vector.tensor_add(out=cqs[:, :], in0=cq[:, 0:1], in1=cq[:, 1:2])
            cq2s = const.tile([D_PER, 1], F32)    # c0 q0^2 + c1 q1^2
            nc.vector.tensor_add(out=cq2s[:, :], in0=cq2[:, 0:1], in1=cq2[:, 1:2])

            # --- bf16 diagonal weight matrices
            _dn = [0]

            def diag(scalar_ap):
                _dn[0] += 1
                t = const.tile([D_PER, D_PER], BF16, tag=f"diag{_dn[0]}")
                nc.vector.tensor_scalar_mul(out=t[:, :], in0=eyesb[:, :],
                                            scalar1=scalar_ap)
                return t

            w_q = [[diag(t[:, n : n + 1]) for n in range(NDIM)] for t in (q, q2, q3)]
            w_cy = [[diag(t[:, n : n + 1]) for n in range(NDIM)]
                    for t in (cc, cq, cq2, cq3)]  # Y-term weights for r=0..3
            w_w = diag(csb[:, 8:9])    # x term of pre_0
            w_cw = diag(csum[:, 0:1])  # x_pr self term, r>=1
            w_cqs = diag(cqs[:, 0:1])
            w_cq2s = diag(cq2s[:, 0:1])

            q4b = [q4[:, n : n + 1].to_broadcast([D_PER, J]) for n in range(NDIM)]

            # prefetch all slabs; u-blocks in their own (earlier) transfers so
            # the first matmuls are gated by a 1MB DMA instead of 1.75MB
            xus = []
            for b in range(BSZ):
                xu = xup.tile([D_PER, 4, J], BF16, tag="xu")
                nc.sync.dma_start(out=xu[:, :, :], in_=x[:, b, :, :])
                xus.append(xu)

            for b in range(BSZ):
                xu = xus[b]

                # --- u_n in PSUM, Y_n = scan(q_n^4, u_n)
                Y = []
                for n in range(NDIM):
                    pu = psu.tile([D_PER, J], F32, tag="u")
                    for g in range(NG):
                        s = bass.ts(g, CH)
                        # c_n is folded into u: scan output is Y'_n = c_n Y_n
                        nc.tensor.matmul(pu[:, s], w_cy[0][n][:, :], xu[:, 0, s],
                                         start=True, stop=False)
                        for k in range(1, 4):  # + c_n q^k * x[4j-k]
                            if g == 0:
                                nc.tensor.matmul(
                                    pu[:, 1:CH], w_cy[k][n][:, :],
                                    xu[:, 4 - k, 0 : CH - 1],
                                    start=False, stop=(k == 3))
                            else:
                                nc.tensor.matmul(
                                    pu[:, s], w_cy[k][n][:, :],
                                    xu[:, 4 - k, g * CH - 1 : (g + 1) * CH - 1],
                                    start=False, stop=(k == 3))
                    yn = yp.tile([D_PER, J], BF16, tag=f"y{n}")
                    nc.vector.tensor_tensor_scan(
                        out=yn[:, :], data0=q4b[n], data1=pu[:, :],
                        initial=0.0, op0=ALU.mult, op1=ALU.add,
                    )
                    Y.append(yn)

                # --- outputs: pre_r accumulated in PSUM, silu evacuates
                ob = op.tile([D_PER, DEC, J], BF16)
                for pair in (1, 0):  # heavy phase-pair (2,3) first: lighter tail
                    for g in range(NG):
                        s = bass.ts(g, CH)
                        pt = psc.tile([D_PER, 2 * CH], F32, tag="cmb")
                        for h in range(2):
                            r = 2 * pair + h
                            tgt = pt[:, bass.ts(h, CH)]
                            if r == 0:
                                ysum = ysp.tile([D_PER, CH], BF16, tag="ys")
                                nc.vector.tensor_add(out=ysum[:, :],
                                                     in0=Y[0][:, s], in1=Y[1][:, s])
                                nc.tensor.matmul(tgt, eyesb[:, :], ysum[:, :],
                                                 start=True, stop=False)
                            else:
                                nc.tensor.matmul(tgt, w_q[r - 1][0][:, :], Y[0][:, s],
                                                 start=True, stop=False)
                                nc.tensor.matmul(tgt, w_q[r - 1][1][:, :], Y[1][:, s],
                                                 start=False, stop=False)
                            # x terms: phase r block is xc[r-1] (r>=1), xu[0] for r=0
                            xw = [(w_w, None) if r == 0 else (w_cw, r)]
                            if r == 2:
                                xw.append((w_cqs, 1))
                            elif r == 3:
                                xw.append((w_cqs, 2))
                                xw.append((w_cq2s, 1))
                            for i, (wt, rr) in enumerate(xw):
                                rhs = xu[:, 0, s] if rr is None else xu[:, rr, s]
                                nc.tensor.matmul(tgt, wt[:, :], rhs,
                                                 start=False, stop=(i == len(xw) - 1))
                        # silu: pt[:, h*512 + k] -> ob[:, 2*pair + h, 512g + k]
                        in_ap = pt[:, :].rearrange("p (h k) -> p h k", h=2)
                        nc.scalar.activation(
                            out=ob[:, 2 * pair : 2 * pair + 2, s],
                            in_=in_ap, func=AF.Silu)
                    # stream this phase-pair out while the next pair computes
                    nc.sync.dma_start(
                        out=out[:, b, 2 * pair : 2 * pair + 2, :],
                        in_=ob[:, 2 * pair : 2 * pair + 2, :])

    nc.compile()
    return nc


_CACHE: dict = {}


def _get_nc():
    if "nc" not in _CACHE:
        _CACHE["nc"] = build_bass()
    return _CACHE["nc"]


def make_in_maps(inputs):
    x = np.asarray(inputs["x"], np.float32)
    delta = np.asarray(inputs["delta"], np.float32).reshape(EMBED_DIM, NDIM)
    alpha = np.asarray(inputs["alpha"], np.float32).reshape(EMBED_DIM, NDIM)
    beta = np.asarray(inputs["beta"], np.float32).reshape(EMBED_DIM, NDIM)
    gamma = np.asarray(inputs["gamma"], np.float32).reshape(EMBED_DIM, NDIM)
    omega = np.asarray(inputs["omega"], np.float32).reshape(EMBED_DIM, 1)
    coef_full = np.concatenate([delta, alpha, beta, gamma, omega], axis=1)
    eye = np.eye(D_PER, dtype=ml_dtypes.bfloat16)
    in_maps = []
    for c in range(N_CORES):
        sl = slice(c * D_PER, (c + 1) * D_PER)
        xc = x[:, :, sl].transpose(2, 1, 0).astype(ml_dtypes.bfloat16)  # [128,B,L]
        ph = xc.reshape(D_PER, BSZ, J, DEC).transpose(0, 1, 3, 2)  # [128,B,4,J]
        in_maps.append(
            {"x": np.ascontiguousarray(ph),
             "coef": np.ascontiguousarray(coef_full[sl]), "eye": eye}
        )
    return in_maps


def gather_out(results):
    out = np.empty((SEQ_LEN, BSZ, EMBED_DIM), np.float32)
    for c in range(N_CORES):
        # [128, B, 4, J] phase-major -> [l = 4j+r, b, d]
        arr = results[c]["out"].astype(np.float32)
        out[:, :, c * D_PER : (c + 1) * D_PER] = arr.transpose(3, 2, 1, 0).reshape(
            SEQ_LEN, BSZ, D_PER
        )
    return out


def _run(inputs, **kwargs):
    nc = _get_nc()
    in_maps = make_in_maps(inputs)
    res = run_bass_kernel_spmd(nc, in_maps, core_ids=list(range(N_CORES)), **kwargs)
    return gather_out(res.results), res


def kernel(**inputs) -> np.ndarray:
    out, _ = _run(inputs)
    return out



# revision 4
# speedup vs baseline: 1.1445x; 1.1445x over previous
"""MultiHeadEMA on 8 Trainium2 NeuronCores — v2 (scheduling-optimized).

Strategy
--------
Channel-sharded: embed_dim=1024 -> 8 slices of 128 channels, one per core.
The FFT conv is an order-2 IIR  y_n[l] = q_n y_n[l-1] + x[l],
out = silu(c0 y0 + c1 y1 + omega x), decimated by 4 for the DVE scan:
    Y_n[j] = q_n^4 Y_n[j-1] + u_n[j],  u_n[j] = sum_k c_n q_n^k x[4j-k]
u_n is accumulated by diagonal matmuls (tensor engine, bf16) into PSUM; the
scan reads PSUM at 1/4 length. Output phases expand into diagonal matmuls
over (Y0, Y1, x phase blocks) accumulated in PSUM, evacuated by Silu.

v2 changes vs baseline (68.9us):
- All 17 diagonal weight matrices + scalar tables precomputed on the host
  and DMA'd in, killing the ~12us device-side coefficient/diag ramp.
- r=0 phase assembled on DVE (ysum = Y0'+Y1' tensor add, then
  scalar_tensor_tensor x0*w + ysum), saving 2 matmul units/batch on PE.
- Shared-weight x-tap matmuls merged into one 2-block-rhs matmul
  (csum on (x2,x3), cqs on (x1,x2)) — fewer LDWEIGHTS.
- Software pipeline: PE stream is u(0), u(1), out(0), u(2), out(1), ... so
  scans(b) (DVE) run under out(b-1) matmuls and PE never waits on a scan.
- PE p-state warmup dummies; batch-0 x DMA split per phase block in tap
  order; outputs DMA'd from the ACT engine's DGE queue (inputs on sync's).
"""

import numpy as np
import ml_dtypes

import concourse.bass as bass
import concourse.bacc as bacc
import concourse.tile as tile
from concourse import mybir
from concourse.bass_utils import run_bass_kernel_spmd

SEQ_LEN, BSZ, EMBED_DIM, NDIM = 4096, 4, 1024, 2
N_CORES = 8
D_PER = EMBED_DIM // N_CORES  # 128 channels/core
SCALE = (1.0 / NDIM) ** 0.5
DEC = 4
J = SEQ_LEN // DEC            # 1024
CH = 512                      # matmul chunk / fp32 PSUM bank
NG = J // CH                  # 2
F32 = mybir.dt.float32
BF16 = mybir.dt.bfloat16
AF = mybir.ActivationFunctionType
ALU = mybir.AluOpType

# weight-table layout: wtab[:, i, :] is diag matrix i (lhsT layout)
#  0..7   u-taps:      w_u[n][k] = c_n q_n^k         (n*4 + k)
#  8..13  Y-terms:     w_y[n][r] = q_n^r, r=1..3     (8 + n*3 + (r-1))
#  14     csum = c0 + c1 + omega
#  15     cqs  = c0 q0 + c1 q1
#  16     cq2s = c0 q0^2 + c1 q1^2
NW = 17
IW_U = lambda n, k: n * 4 + k
IW_Y = lambda n, r: 8 + n * 3 + (r - 1)
IW_CW, IW_CQS, IW_CQ2S = 14, 15, 16


def build_bass():
    nc = bacc.Bacc(name="multihead_ema_v2")
    x = nc.dram_tensor("x", [D_PER, BSZ, DEC, J], BF16, kind="ExternalInput")
    wtab = nc.dram_tensor("wtab", [D_PER, NW, D_PER], BF16, kind="ExternalInput")
    # scal columns: [q0^4, q1^4, omega]
    scal = nc.dram_tensor("scal", [D_PER, 3], F32, kind="ExternalInput")
    out = nc.dram_tensor("out", [D_PER, BSZ, DEC, J], BF16, kind="ExternalOutput")

    with tile.TileContext(nc) as tc:
        with (
            tc.tile_pool(name="const", bufs=1) as const,
            tc.tile_pool(name="xup", bufs=4) as xup,
            tc.tile_pool(name="yp", bufs=2) as yp,
            tc.tile_pool(name="vp", bufs=2) as vp,
            tc.tile_pool(name="op", bufs=2) as op,
            tc.tile_pool(name="psu", bufs=1, space="PSUM") as psu,
            tc.tile_pool(name="pso", bufs=1, space="PSUM") as pso,
        ):
            # --- input DMAs on the sync DGE queue, in consumption order
            wsb = const.tile([D_PER, NW, D_PER], BF16)
            nc.sync.dma_start(out=wsb[:, :, :], in_=wtab[:, :, :])
            ssb = const.tile([D_PER, 3], F32)
            nc.sync.dma_start(out=ssb[:, :], in_=scal[:, :])

            xus = []
            for b in range(BSZ):
                xu = xup.tile([D_PER, DEC, J], BF16, tag="xu")
                if b == 0:
                    # per-phase DMAs in u-tap consumption order: x0,x3,x2,x1
                    for blk in (0, 3, 2, 1):
                        nc.sync.dma_start(out=xu[:, blk, :], in_=x[:, 0, blk, :])
                else:
                    nc.sync.dma_start(out=xu[:, :, :], in_=x[:, b, :, :])
                xus.append(xu)

            W = [wsb[:, i, :] for i in range(NW)]
            q4b = [ssb[:, n : n + 1].to_broadcast([D_PER, J]) for n in range(NDIM)]

            u_tiles = [None, None]  # PSUM [128, J] fp32 per n (tags u0,u1)
            Y = {}                  # (b, n) -> SBUF bf16 [128, J]
            p0 = {}                 # b -> SBUF bf16 [128, J] (pre_0)

            def emit_u_and_scans(b):
                xu = xus[b]
                for n in range(NDIM):
                    pu = psu.tile([D_PER, J], F32, tag=f"u{n}")
                    u_tiles[n] = pu
                    if b == 0 and n == 0:
                        # p-state warmup: harmless dummies into the same bank
                        for _ in range(6):
                            nc.tensor.matmul(pu[:, 0:CH], W[0], wsb[:, 0:4, :],
                                             start=True, stop=True,
                                             skip_group_check=True)
                    for g in range(NG):
                        s = bass.ts(g, CH)
                        nc.tensor.matmul(pu[:, s], W[IW_U(n, 0)], xu[:, 0, s],
                                         start=True, stop=False)
                        for k in range(1, 4):
                            if g == 0:
                                nc.tensor.matmul(
                                    pu[:, 1:CH], W[IW_U(n, k)],
                                    xu[:, 4 - k, 0 : CH - 1],
                                    start=False, stop=(k == 3))
                            else:
                                nc.tensor.matmul(
                                    pu[:, s], W[IW_U(n, k)],
                                    xu[:, 4 - k, g * CH - 1 : (g + 1) * CH - 1],
                                    start=False, stop=(k == 3))
                    yn = yp.tile([D_PER, J], BF16, tag=f"y{n}")
                    nc.vector.tensor_tensor_scan(
                        out=yn[:, :], data0=q4b[n], data1=pu[:, :],
                        initial=0.0, op0=ALU.mult, op1=ALU.add)
                    Y[(b, n)] = yn
                # r0 on DVE: pre0 = w*x0 + (Y0 + Y1)
                ys = vp.tile([D_PER, J], BF16, tag="ys")
                nc.vector.tensor_tensor(out=ys[:, :], in0=Y[(b, 0)][:, :],
                                        in1=Y[(b, 1)][:, :], op=ALU.add)
                pz = vp.tile([D_PER, J], BF16, tag="p0")
                nc.vector.scalar_tensor_tensor(
                    out=pz[:, :], in0=xu[:, 0, :], scalar=ssb[:, 2:3],
                    in1=ys[:, :], op0=ALU.mult, op1=ALU.add)
                p0[b] = pz

            def emit_outputs(b):
                xu = xus[b]
                Y0, Y1 = Y[(b, 0)], Y[(b, 1)]
                ob = op.tile([D_PER, DEC, J], BF16)
                # silu(pre0) from SBUF
                nc.scalar.activation(out=ob[:, 0, :], in_=p0[b][:, :], func=AF.Silu)

                # r2/r3 pair, g=0
                def r23(g):
                    s = bass.ts(g, CH)
                    pt = pso.tile([D_PER, 2, CH], F32, tag="p23")
                    for h, r in ((0, 2), (1, 3)):
                        nc.tensor.matmul(pt[:, h, :], W[IW_Y(0, r)], Y0[:, s],
                                         start=True, stop=False)
                        nc.tensor.matmul(pt[:, h, :], W[IW_Y(1, r)], Y1[:, s],
                                         start=False, stop=False)
                    # x taps: r2 <- cqs*x1 + csum*x2 ; r3 <- cq2s*x1 + cqs*x2 + csum*x3
                    nc.tensor.matmul(pt[:, 0, :], W[IW_CQS], xu[:, 1, s],
                                     start=False, stop=False)
                    nc.tensor.matmul(pt[:, 0, :], W[IW_CW], xu[:, 2, s],
                                     start=False, stop=True)
                    nc.tensor.matmul(pt[:, 1, :], W[IW_CQ2S], xu[:, 1, s],
                                     start=False, stop=False)
                    nc.tensor.matmul(pt[:, 1, :], W[IW_CQS], xu[:, 2, s],
                                     start=False, stop=False)
                    nc.tensor.matmul(pt[:, 1, :], W[IW_CW], xu[:, 3, s],
                                     start=False, stop=True)
                    nc.scalar.activation(out=ob[:, 2:4, s], in_=pt[:, :, :],
                                         func=AF.Silu)

                r23(0)
                # r1 (both g) between the two r23 groups so silu(p23 g0) can drain
                p1 = pso.tile([D_PER, J], F32, tag="p1")
                for g in range(NG):
                    s = bass.ts(g, CH)
                    nc.tensor.matmul(p1[:, s], W[IW_Y(0, 1)], Y0[:, s],
                                     start=True, stop=False)
                    nc.tensor.matmul(p1[:, s], W[IW_Y(1, 1)], Y1[:, s],
                                     start=False, stop=False)
                    nc.tensor.matmul(p1[:, s], W[IW_CW], xu[:, 1, s],
                                     start=False, stop=True)
                nc.scalar.activation(out=ob[:, 1, :], in_=p1[:, :], func=AF.Silu)
                # first half out: r0 + r1 (ACT DGE queue)
                nc.scalar.dma_start(out=out[:, b, 0:2, :], in_=ob[:, 0:2, :])
                r23(1)
                nc.scalar.dma_start(out=out[:, b, 2:4, :], in_=ob[:, 2:4, :])

            # software pipeline: u(0), u(1), out(0), u(2), out(1), u(3), out(2), out(3)
            emit_u_and_scans(0)
            for b in range(1, BSZ):
                emit_u_and_scans(b)
                emit_outputs(b - 1)
            emit_outputs(BSZ - 1)

    nc.compile()
    return nc


_CACHE: dict = {}


def _get_nc():
    if "nc" not in _CACHE:
        _CACHE["nc"] = build_bass()
    return _CACHE["nc"]


def make_in_maps(inputs):
    x = np.asarray(inputs["x"], np.float32)
    delta = np.asarray(inputs["delta"], np.float32).reshape(EMBED_DIM, NDIM)
    alpha = np.asarray(inputs["alpha"], np.float32).reshape(EMBED_DIM, NDIM)
    beta = np.asarray(inputs["beta"], np.float32).reshape(EMBED_DIM, NDIM)
    gamma = np.asarray(inputs["gamma"], np.float32).reshape(EMBED_DIM, NDIM)
    omega = np.asarray(inputs["omega"], np.float32).reshape(EMBED_DIM)

    p = 1.0 / (1.0 + np.exp(-delta))
    q = 1.0 - p / (1.0 + np.exp(-alpha))          # [D, N]
    c = p * beta * gamma * SCALE                  # [D, N]
    q4 = q ** 4
    csum = c.sum(1) + omega
    cqs = (c * q).sum(1)
    cq2s = (c * q * q).sum(1)

    # weight diag tables [D, NW] of per-channel values
    wvals = np.zeros((EMBED_DIM, NW), np.float32)
    for n in range(NDIM):
        for k in range(4):
            wvals[:, IW_U(n, k)] = c[:, n] * q[:, n] ** k
        for r in (1, 2, 3):
            wvals[:, IW_Y(n, r)] = q[:, n] ** r
    wvals[:, IW_CW] = csum
    wvals[:, IW_CQS] = cqs
    wvals[:, IW_CQ2S] = cq2s

    in_maps = []
    idx = np.arange(D_PER)
    for cix in range(N_CORES):
        sl = slice(cix * D_PER, (cix + 1) * D_PER)
        xc = x[:, :, sl].transpose(2, 1, 0).astype(ml_dtypes.bfloat16)  # [128,B,L]
        ph = xc.reshape(D_PER, BSZ, J, DEC).transpose(0, 1, 3, 2)       # [128,B,4,J]
        wt = np.zeros((D_PER, NW, D_PER), dtype=ml_dtypes.bfloat16)
        wt[idx, :, idx] = wvals[sl].astype(ml_dtypes.bfloat16)
        sc = np.stack([q4[sl, 0], q4[sl, 1], omega[sl]], axis=1).astype(np.float32)
        in_maps.append({
            "x": np.ascontiguousarray(ph),
            "wtab": np.ascontiguousarray(wt),
            "scal": np.ascontiguousarray(sc),
        })
    return in_maps


def gather_out(results):
    out = np.empty((SEQ_LEN, BSZ, EMBED_DIM), np.float32)
    for c in range(N_CORES):
        arr = results[c]["out"].astype(np.float32)   # [128, B, 4, J]
        out[:, :, c * D_PER : (c + 1) * D_PER] = arr.transpose(3, 2, 1, 0).reshape(
            SEQ_LEN, BSZ, D_PER)
    return out


def _run(inputs, **kwargs):
    nc = _get_nc()
    in_maps = make_in_maps(inputs)
    res = run_bass_kernel_spmd(nc, in_maps, core_ids=list(range(N_CORES)), **kwargs)
    return gather_out(res.results), res


def kernel(**inputs) -> np.ndarray:
    out, _ = _run(inputs)
    return out
